# revision 86
# baseline (speedup 1.0000x reference)
"""Trainium2 Bass kernel for nn_MultiHeadAttention_68959994904763.

Sharding (8 NeuronCores): 2-D tensor-parallel — batch (2) x head-groups (4).
Core c handles batch b = c // 4 and heads [4g, 4g+4) with g = c % 4.
Each core computes a partial output o_heads @ W_o for its 4 heads; the
host sums the 4 (bf16) partials per batch and adds the (host-folded)
bias b_o_eff = b_v.flatten() @ W_o + b_o.  All layout prep (x transpose,
weight pair-stacking/reshape, mask generation) is host-side; all FLOPs
(projections, attention, output projection) run on device.

All matmuls in bf16 (tolerance is 2e-2; bf16 end-to-end lands 3.8e-3):
  1. x^T pre-transposed on host, DMA'd bf16 as [128, 2048] chunk tiles
     (xT[0] in halves).  Merged small inputs (biases [P,4] f32;
     masks+identity [P, 4*NQ+P] bf16) — each dma_start costs ~0.6us of
     serialized Sync dispatch, so few, big DMAs, in consumption order.
  2. ALL QKV projection work runs as q/k window chains and v s-chunk
     chains; only the two window-0 pair-0 chains run inline as the
     prefix (first PE matmul ~8.5us; first scores ~15us), everything
     else is FIFO filler woven into the attention pass.  v lands
     DIRECTLY in per-(head, s-chunk) v_aug [128, 65] tiles (col 64 =
     memset ones — the softmax-denominator column).
  3. Scores s^T[k, q] per (head, q-window 512, k-chunk 128), causal
     tiles only, diagonal tiles column-trimmed; exp on ACT from PSUM
     (no max subtraction: |score| <= ~3 here); diagonal tiles masked
     by 0/1 bf16 masks on DVE.
  4. o-chains in [q, d] ORIENTATION: stationary = 128-q-col slice of
     the exp tile, moving = v_aug [128, 65].  Output po[q, qc*128+c]
     holds numerator cols 0:64 and the denominator at col 64 — PER
     PARTITION, so normalize = one strided DVE reciprocal
     (po[:, 64::128] -> [128,4]) + 4 per-partition tensor_scalar_mul
     into bf16 oQ [128 q, 128 pair-d] tiles.  This kills the old
     epilogue entirely (was: ACT drow copy + PE ones-broadcast + 64x
     0.9us DVE [64,128] reciprocals = ~58us DVE).
  5. After both heads of a pair: 4 PE transposes (plain matmul vs
     identity moving — exact x1.0, f32 accum) flip oQ -> oT [d, q],
     DVE-copied to bf16.  W_o accumulates the 2 pairs per [s-chunk,
     512] tile; PSUM->bf16 copies on DVE for the first-processed two
     windows, on ACT for the last two (popped after the exp stream
     dries up, when ACT is the idle engine).  Output DMA'd bf16
     per-pair; host upcasts and sums.
  6. Burst schedule: per (window, head) all nkc score matmuls in
     chunks of 4; after each chunk, pop 7 deferred PE-work thunks from
     the FIFO (QKV chains, previous head's o-chain in <=4-matmul
     per-qc thunks, norm, transposes, previous pr1-window's W_o).
     pr0 pass ascends windows (window 0 needs only the prefix, so exp
     starts early); pr1 descends (the drain tail carries the 4-chunk
     window).  PSUM: ps_s=3, ps_o=2, ps_wo=3 (qkv chains + W_o + pt
     share ps_wo — it was the contended pool at 2).  SBUF pool depths
     matter a lot: epool(es)=2, oq=4, obuf=8, rpool=3 bought ~8us of
     WAR-serialization relief over 1/2/4/2.

Measured on trn2 (8 cores, NTFF): 158-162us, rel err 3.8e-3 (prior
session's [d,q]-orientation kernel: 201-204us; f32r: ~327us).
Run-to-run noise is ~±2us, and the device occasionally throttles ~12%
for minutes (ALL engine busy times inflate uniformly — check
TensorMatrix busy ~121-126us = healthy before trusting a comparison).

HW findings that shaped this (beyond the docstring of the previous
session, which still holds for PE p-state/shape-uniformity, scheduler
reordering, reciprocal_approx_fast being broken, and engine costs):
  - PSUM accumulation groups must be CONTIGUOUS per bank: interleaving
    start/stop groups of different column regions in one bank silently
    drops all pre-stop partials (each region ends up = its last
    contribution).  Different banks interleave fine.
  - All-stationary-[128,128] streams hold full PE clock even when
    moving widths mix (512/128/65): 512-col matmuls run at the 262ns
    floor, 65-col at ~35ns.  The old va zero-padding to [128,128] was
    only needed when va was stationary.
  - ACT activation with a per-partition scale AP (Identity+scale) is
    catastrophically slow (looks like a table reload per call) — use
    DVE tensor_scalar_mul for per-partition scaling.  Identity+bias AP
    is fine (~650ns/[128,512]).
  - DVE tensor_scalar_* with a [P,1] AP from PSUM input costs ~0.75us
    per [128,512] (no 16-bit bypass); ACT Identity+bias is cheaper —
    route by which engine has slack in that phase of the pass.
  - Step-sliced APs (po[:, 64::128]) work on DVE ops.
  - dma_start from nc.gpsimd has high dispatch latency (slower input
    streaming); nc.scalar dispatches sit in ACT program order (a DMA
    emitted before the attention section delays the first bias-adds).
    Single Sync-ring streaming of the 4MB xT (~22us) is what the
    early-pass FIFO pops wait on — unsolved, costs ~4-6us of early PE
    stalls.
  - fp8 is a dead end for accuracy here: for random-walk sums
    (|sum| ~ rms*sqrt(N)) per-element fp8e4m3 noise does NOT average
    away — projections would carry ~12% error.
"""

import os
import sys
import types

import numpy as np

S, E, D = 2048, 1024, 64
P = 128
NQ = 512  # q-window (moving operand) size
SC = S // P  # 16 s-chunks
EC = E // P  # 8 e-chunks
QW = S // NQ  # 4 q-windows
N_CORES = 8


def _ensure_axon_hooks():
    """Provide antenv.axon_hooks (NTFF profile hook registry) if the image
    lacks it, and register the ctypes-based hook so trace=True works."""
    try:
        from antenv.axon_hooks import get_axon_ntff_profile_hook  # noqa: F401
        return
    except ImportError:
        pass
    import antenv

    mod = types.ModuleType("antenv.axon_hooks")
    _h = [None]
    mod.set_axon_ntff_profile_hook = lambda h: _h.__setitem__(0, h)
    mod.get_axon_ntff_profile_hook = lambda: _h[0]
    sys.modules["antenv.axon_hooks"] = mod
    antenv.axon_hooks = mod
    try:
        from trn_agent_boot.trn_boot import _ntff_profile_via_ctypes

        so_path = "/opt/axon/libaxon_pjrt.so"
        if os.path.exists(so_path):
            mod.set_axon_ntff_profile_hook(_ntff_profile_via_ctypes(so_path))
    except Exception:
        pass


def _build_program():
    import concourse.bass as bass  # noqa: F401
    import concourse.mybir as mybir
    import concourse.tile as tile
    from concourse import bacc
    import contextlib

    f32 = mybir.dt.float32
    bf16 = mybir.dt.bfloat16

    nc = bacc.Bacc("TRN2", target_bir_lowering=False, debug=False)

    xT_d = nc.dram_tensor("xT", [E, S], bf16, kind="ExternalInput").ap()
    wq_d = nc.dram_tensor("wq", [2, P, E], bf16, kind="ExternalInput").ap()
    wk_d = nc.dram_tensor("wk", [2, P, E], bf16, kind="ExternalInput").ap()
    wv_d = nc.dram_tensor("wv", [2, P, E], bf16, kind="ExternalInput").ap()
    # biases merged into one [P, 4] f32 tensor (cols: bq0, bk0, bq1, bk1)
    # and masks+ident into one [P, 4*NQ+P] bf16 tensor: each dma_start
    # costs ~0.6us of serialized Sync dispatch, so fewer, bigger loads
    bias_d = nc.dram_tensor("biases", [P, 4], f32, kind="ExternalInput").ap()
    wo_d = nc.dram_tensor("wo", [2, P, E], bf16, kind="ExternalInput").ap()
    mi_d = nc.dram_tensor("mi", [P, 4 * NQ + P], bf16, kind="ExternalInput").ap()
    out_d = nc.dram_tensor("out", [S, E], bf16, kind="ExternalOutput").ap()

    Act = mybir.ActivationFunctionType

    with tile.TileContext(nc) as tc:
        # Forced scheduling order: the tile scheduler dispatches by its own
        # cost-model simulation and freely reorders per-engine streams; its
        # model is missing the PE stationary-shape-switch penalty (~115ns)
        # and the p-state clock ramp, so its interleavings run ~2x slow on
        # HW.  bass_wait_until_ts floors are scheduler-sim-only (no hardware
        # waits), so monotonically increasing floors pin per-engine issue
        # order to emission order.
        # NOTE: forcing order via tc.tile_set_cur_wait() floors produced
        # deterministic data corruption (the scheduler appears to rely on
        # sim-time proximity for sync/allocation decisions) — do not use.
        def tick():
            pass

        with contextlib.ExitStack() as top:
            persist = top.enter_context(tc.tile_pool(name="persist", bufs=1))

            # --- persistent constants / weights ---
            # one [P, 4] f32 bias tile (cols: bq0, bk0, bq1, bk1)
            bias_t = persist.tile([P, 4], f32, tag="bias", name="bias")
            bq_t = [bias_t[:, 2 * pr : 2 * pr + 1] for pr in range(2)]
            bk_t = [bias_t[:, 2 * pr + 1 : 2 * pr + 2] for pr in range(2)]

            def load_biases():
                nc.sync.dma_start(bias_t[:], bias_d)

            # persistent activations.  kT is stored PER HEAD, zero-padded to
            # the full 128 partitions (other head's rows = 0), and va is
            # zero-padded to [128, 128]: this makes scores and o-matmuls the
            # SAME 128x128-stationary shape as QKV/W_o — the PE p-state only
            # ramps to full clock on shape-uniform instruction streams
            # (mixed-shape alternation pins it at ~1.2-1.4 GHz).
            qT = [persist.tile([P, S], bf16, tag=f"qT{pr}", name=f"qT{pr}") for pr in range(2)]
            kTh = [persist.tile([P, S], bf16, tag=f"kTh{h}", name=f"kTh{h}") for h in range(4)]
            oT = [persist.tile([P, S], bf16, tag=f"oT{pr}", name=f"oT{pr}") for pr in range(2)]
            for h in range(4):
                # zero the rows belonging to the other head of the pair
                if h % 2:
                    nc.vector.memset(kTh[h][0:D, :], 0.0)
                else:
                    nc.vector.memset(kTh[h][D:P, :], 0.0)
            # v_aug per (head, s-chunk): [128, 65], col 64 = 1.0 (softmax-
            # denominator ones column).  Only the 65-col MOVING operand of
            # the o-matmuls now, so no zero-padding to 128 cols needed.
            va = [
                [persist.tile([P, D + 1], bf16, tag=f"va{h}_{sc}", name=f"va{h}_{sc}") for sc in range(SC)]
                for h in range(4)
            ]
            for h in range(4):
                for sc in range(SC):
                    nc.vector.memset(va[h][sc][:, D : D + 1], 1.0)
            # masks + identity in one [P, 4*NQ+P] bf16 tile (one DMA):
            # mask j at cols [j*NQ, (j+1)*NQ), PE-transpose identity at
            # cols [4*NQ, 4*NQ+P)
            mi_t = persist.tile([P, 4 * NQ + P], bf16, tag="mi", name="mi")
            mask_t = [mi_t[:, j * NQ : (j + 1) * NQ] for j in range(4)]
            ident = mi_t[:, 4 * NQ : 4 * NQ + P]

            # ---------- input DMA ----------
            xTp = top.enter_context(tc.tile_pool(name="xT", bufs=1))
            xT = [xTp.tile([P, S], bf16, tag=f"xT{ec}", name=f"xT{ec}") for ec in range(EC)]
            wpool = top.enter_context(tc.tile_pool(name="wqkv", bufs=1))

            # DMA issue order matters: later DMAs queue behind earlier
            # ones, so emit in consumption order — the attention pass
            # starts with window-0 scores after just the q0/k0 window-0
            # projection chains, so: wq0, xT0, wk0, biases, then masks +
            # ident (first diagonal window needs them ~immediately), then
            # the rest of xT, then pair-1 weights, wo.
            wq_t, wk_t, wv_t = [], [], []

            def load_wkind(nm, store, dram, pr):
                t = wpool.tile([P, E], bf16, tag=f"w{nm}{pr}", name=f"w{nm}{pr}")
                nc.sync.dma_start(t[:], dram[pr])
                store.append(
                    [t[:, ec * P : (ec + 1) * P] for ec in range(EC)]
                )

            # xT[0] split in halves: the prefix chains only need window-0
            # columns, so the first piece (256KB) unblocks the PE ~1.4us
            # sooner than one 512KB DMA would
            # xT rides the idle GpSimd and Vector DMA rings, in parallel
            # with the weights on the Sync ring: the projection chains
            # consume all 4MB of xT within ~20us of start, which one
            # ~185GB/s ring cannot deliver.  Chunks alternate rings so
            # arrival order tracks consumption order (ec ascending).
            load_wkind("q", wq_t, wq_d, 0)
            nc.sync.dma_start(xT[0][:, 0 : S // 2], xT_d[0:P, 0 : S // 2])
            load_wkind("k", wk_t, wk_d, 0)
            load_biases()
            nc.sync.dma_start(xT[0][:, S // 2 : S], xT_d[0:P, S // 2 : S])
            for ec in range(1, EC):
                nc.sync.dma_start(xT[ec][:], xT_d[ec * P : (ec + 1) * P, :])
            nc.sync.dma_start(mi_t[:], mi_d)
            load_wkind("v", wv_t, wv_d, 0)
            load_wkind("q", wq_t, wq_d, 1)
            load_wkind("k", wk_t, wk_d, 1)
            load_wkind("v", wv_t, wv_d, 1)
            wo_t = []
            for pr in range(2):
                t = persist.tile([P, E], bf16, tag=f"wo{pr}", name=f"wo{pr}")
                nc.sync.dma_start(t[:], wo_d[pr])
                wo_t.append(t)

            # ---------- attention + W_o (single fused pass) ----------
            # Burst schedule: per (window, head) emit all nkc score matmuls
            # in chunks of 4; after each chunk, pop deferred PE work (QKV
            # projection chains, the PREVIOUS head's o-chain, its norm, the
            # window transposes, W_o pairs of the previous pr1 window) from
            # a FIFO.  Scores pace to ACT exp, and the popped work fills
            # the PE slack without the per-instruction score/o interleave
            # penalty (measured +80%/matmul when strictly alternating).
            with contextlib.ExitStack() as ph34:
                ps_s = ph34.enter_context(
                    tc.tile_pool(name="ps_s", bufs=3, space="PSUM")
                )
                ps_o = ph34.enter_context(
                    tc.tile_pool(name="ps_o", bufs=2, space="PSUM")
                )
                ps_wo = ph34.enter_context(
                    tc.tile_pool(name="ps_wo", bufs=3, space="PSUM")
                )
                epool = ph34.enter_context(tc.tile_pool(name="epool", bufs=2))
                rpool = ph34.enter_context(tc.tile_pool(name="rpool", bufs=3))
                obuf = ph34.enter_context(tc.tile_pool(name="obuf", bufs=8))
                # normalized per-window o in [q, pair-dims] orientation,
                # double-buffered across windows
                oqpool = ph34.enter_context(tc.tile_pool(name="oq", bufs=4))

                fifo = []  # deferred PE-work thunks, popped between chunks
                last_trs = []  # final window's transposes, interleaved
                # with its W_o pairs in the drain

                def wo_pair(qw, i, n):
                    def t():
                        tick()
                        sc = qw * (NQ // P) + i
                        pw = ps_wo.tile([P, NQ], f32, tag="pwo", name="pw")
                        for step, pr in enumerate((0, 1)):
                            nc.tensor.matmul(
                                pw[:],
                                oT[pr][:, sc * P : (sc + 1) * P],
                                wo_t[pr][:, n * NQ : (n + 1) * NQ],
                                start=(step == 0),
                                stop=(step == 1),
                            )
                        ob = obuf.tile([P, NQ], bf16, tag="ob", name="ob")
                        # window-3 pairs pop while ACT is still exp-busy;
                        # later windows' pairs pop after the exp stream has
                        # dried up, where ACT is the idle engine
                        if qw >= QW - 2:
                            nc.vector.tensor_copy(ob[:], pw[:])
                        else:
                            nc.scalar.copy(ob[:], pw[:])
                        nc.sync.dma_start(
                            out_d[sc * P : (sc + 1) * P, n * NQ : (n + 1) * NQ],
                            ob[:],
                        )
                    return t

                # ALL QKV projection work runs as q/k window chains and v
                # s-chunk chains on the ps_wo pool (bufs=2 rotation): two
                # chains (q0/k0 window 0) run inline as the prefix — just
                # enough for the first score burst — and everything else
                # is FIFO filler woven into the attention pass.
                def qkv_qk(pr, kind, sw):
                    def t():
                        tick()
                        w_t = wq_t[pr] if kind == "q" else wk_t[pr]
                        b_t = bq_t[pr] if kind == "q" else bk_t[pr]
                        pq = ps_wo.tile([P, NQ], f32, tag="pwo", name="pq1")
                        for ec in range(EC):
                            nc.tensor.matmul(
                                pq[:],
                                w_t[ec][:],
                                xT[ec][:, sw * NQ : (sw + 1) * NQ],
                                start=(ec == 0),
                                stop=(ec == EC - 1),
                            )
                        # bias-add + PSUM->SBUF evacuation: pair-0 chains
                        # run while ACT is still exp-idle (ACT Identity);
                        # pair-1 chains pop mid-pass where ACT is the
                        # bottleneck, so they evacuate on DVE instead
                        if kind == "q":
                            if pr == 0:
                                nc.scalar.activation(
                                    qT[pr][:, sw * NQ : (sw + 1) * NQ],
                                    pq[:],
                                    Act.Identity,
                                    bias=b_t[:],
                                )
                            else:
                                nc.vector.tensor_scalar_add(
                                    qT[pr][:, sw * NQ : (sw + 1) * NQ], pq[:], b_t[:]
                                )
                        else:
                            for hh in range(2):
                                o0 = hh * D
                                if pr == 0:
                                    nc.scalar.activation(
                                        kTh[2 * pr + hh][o0 : o0 + D, sw * NQ : (sw + 1) * NQ],
                                        pq[o0 : o0 + D, :],
                                        Act.Identity,
                                        bias=b_t[o0 : o0 + D, :],
                                    )
                                else:
                                    nc.vector.tensor_scalar_add(
                                        kTh[2 * pr + hh][o0 : o0 + D, sw * NQ : (sw + 1) * NQ],
                                        pq[o0 : o0 + D, :],
                                        b_t[o0 : o0 + D, :],
                                    )
                    return t

                def qkv_v(pr, sc):
                    def t():
                        tick()
                        pvt = ps_wo.tile([P, NQ], f32, tag="pwo", name="pv1")
                        for ec in range(EC):
                            nc.tensor.matmul(
                                pvt[:, 0:P],
                                xT[ec][:, sc * P : (sc + 1) * P],
                                wv_t[pr][ec][:],
                                start=(ec == 0),
                                stop=(ec == EC - 1),
                            )
                        for hh in range(2):
                            nc.vector.tensor_copy(
                                va[2 * pr + hh][sc][:, 0:D],
                                pvt[:, hh * D : (hh + 1) * D],
                            )
                    return t

                # prefix: just the two window-0 pair-0 chains, inline
                qkv_qk(0, "q", 0)()
                qkv_qk(0, "k", 0)()
                # filler order = consumption order: remaining pair-0 q/k
                # window chains (windows ascend in the pr0 pass), pair-0 v,
                # pair-1 q/k (needed at the pr1 pass), pair-1 v
                fifo.extend(
                    qkv_qk(0, k, sw) for sw in range(1, QW) for k in ("q", "k")
                )
                fifo.extend(qkv_v(0, sc) for sc in range(SC))
                fifo.extend(qkv_qk(1, k, sw) for k in ("q", "k") for sw in range(QW))
                fifo.extend(qkv_v(1, sc) for sc in range(SC))

                # pr0 pass ascends windows (window 0 only needs the prefix
                # chains, so exp starts ~20us earlier); pr1 descends so the
                # drain tail carries the 4-chunk window, not the 16-chunk
                # one.  W_o(w) pops one pr1-window after w is finished.
                for pr_pass in range(2):
                  wo_order = (0, 1, 2, 3) if pr_pass == 0 else (3, 2, 1, 0)
                  for qi, qw in enumerate(wo_order):
                    nkc = 4 * qw + 4  # causal k-chunks for this q-window
                    # per-window normalized o tiles in [q, pair-dims]
                    # orientation: head hh writes cols [hh*64, hh*64+64)
                    oQ = [
                        oqpool.tile([P, P], bf16, tag=f"oq{qc}", name=f"oq{qc}")
                        for qc in range(NQ // P)
                    ]
                    for hh_pass in range(2):
                        h = 2 * pr_pass + hh_pass
                        pr, off = h // 2, (h % 2) * D
                        es = [None] * nkc
                        # scores burst (chunks of 4, popping deferred work)
                        for kc in range(nkc):
                            tick()
                            j = kc - 4 * qw
                            qa = j * P if 0 < j < 4 else 0
                            sl = slice(qa, NQ)
                            ps = ps_s.tile([P, NQ], f32, tag="pss", name="ps")
                            nc.tensor.matmul(
                                ps[:, sl],
                                kTh[h][:, kc * P : (kc + 1) * P],
                                qT[pr][:, qw * NQ + qa : (qw + 1) * NQ],
                                start=True,
                                stop=True,
                                skip_group_check=True,
                            )
                            e = epool.tile(
                                [P, NQ], bf16, tag=f"e{h % 2}_{kc}", name="e"
                            )
                            nc.scalar.activation(e[:, sl], ps[:, sl], Act.Exp)
                            if 0 <= j < 4:
                                nc.vector.tensor_mul(
                                    e[:, sl], e[:, sl], mask_t[j][:, sl]
                                )
                            es[kc] = e
                            if kc % 4 == 3:
                                for t in fifo[:3]:
                                    t()
                                del fifo[:3]
                        po = ps_o.tile([P, NQ], f32, tag="po", name="po")

                        # o-chain in [q, d] orientation: stationary = the
                        # 128-q-col slice of the exp tile, moving = va
                        # [128, 65].  Col 64 of each qc region is the
                        # softmax denominator (per PARTITION now), so the
                        # normalize is a native per-partition tensor_scalar
                        # broadcast — no PE ones-broadcast, no [64,512]
                        # DVE reciprocal, no drow extraction.
                        # Each qc region's accumulation group must be
                        # CONTIGUOUS per PSUM bank (interleaved start/stop
                        # groups in one bank drop the pre-stop partials),
                        # so emit per-qc chains in <=4-matmul thunks.
                        def emit_o(qc, c0, po=po, va_h=va[h], es=es, qw=qw):
                            def t():
                                tick()
                                last = 4 * qw + qc
                                for kc in range(c0, min(c0 + 4, last + 1)):
                                    nc.tensor.matmul(
                                        po[:, qc * P : qc * P + D + 1],
                                        es[kc][:, qc * P : (qc + 1) * P],
                                        va_h[kc][:],
                                        start=(kc == 0),
                                        stop=(kc == last),
                                        skip_group_check=True,
                                    )
                            return t

                        def emit_norm(po=po, oQ=oQ, off=off):
                            def t():
                                tick()
                                rc = rpool.tile([P, NQ // P], f32, tag="rc", name="rc")
                                # one strided reciprocal over the 4
                                # denominator columns (step-slice AP)
                                nc.vector.reciprocal(rc[:], po[:, D :: P])
                                for qc in range(NQ // P):
                                    nc.vector.tensor_scalar_mul(
                                        oQ[qc][:, off : off + D],
                                        po[:, qc * P : qc * P + D],
                                        rc[:, qc : qc + 1],
                                    )
                            return t

                        fifo.extend(
                            emit_o(qc, c0)
                            for qc in range(NQ // P)
                            for c0 in range(0, 4 * qw + qc + 1, 4)
                        )
                        fifo.append(emit_norm())
                        if hh_pass == 1:
                            # both heads' oQ cols done: PE-transpose the 4
                            # [128q, 128d] tiles back to [d, q] via plain
                            # matmul vs identity moving (exact x1.0, f32
                            # accum, keeps 128x128-stationary uniformity),
                            # one thunk per qc so each W_o s-chunk can
                            # start right after its own copy (tail pipelining)
                            ptbox = []  # pt allocated at pop time (qc 0) so
                            # the ps_wo tag rotation follows pop order

                            def emit_tr(qc, pr=pr, qw=qw, oQ=oQ, ptbox=ptbox):
                                def t():
                                    tick()
                                    if qc == 0:
                                        ptbox.append(
                                            ps_wo.tile([P, NQ], f32, tag="pwo", name="pt")
                                        )
                                    pt = ptbox[0]
                                    nc.tensor.matmul(
                                        pt[:, qc * P : (qc + 1) * P],
                                        oQ[qc][:],
                                        ident[:],
                                        start=True,
                                        stop=True,
                                        skip_group_check=True,
                                    )
                                    nc.vector.tensor_copy(
                                        oT[pr][
                                            :, qw * NQ + qc * P : qw * NQ + (qc + 1) * P
                                        ],
                                        pt[:, qc * P : (qc + 1) * P],
                                    )
                                return t
                            if pr_pass == 1 and qi == QW - 1:
                                last_trs.extend(
                                    emit_tr(qc) for qc in range(NQ // P)
                                )
                            else:
                                fifo.extend(
                                    emit_tr(qc) for qc in range(NQ // P)
                                )
                        if pr_pass == 1 and hh_pass == 0 and qi > 0:
                            # previously processed window's W_o: pair-0 oT
                            # rows done in the pair-0 pass; pair-1 transpose
                            # copy queued ahead in the FIFO
                            fifo.extend(
                                wo_pair(wo_order[qi - 1], i, n)
                                for i in range(NQ // P)
                                for n in range(E // NQ)
                            )
                # drain: last head's o-chain + norm, then the last
                # window's transposes interleaved with its W_o pairs
                for t in fifo:
                    t()
                fifo.clear()
                for qc in range(NQ // P):
                    last_trs[qc]()
                    for n in range(E // NQ):
                        wo_pair(wo_order[-1], qc, n)()

    nc._dbg = {
        "qT": qT,
        "kTh": kTh,
        "oT": oT,
        "va": va,
        "ident": ident,
        "oQ_last": oQ,
    }
    nc.compile()
    return nc


def _host_shard(x, W_q, b_q, W_k, b_k, W_v, b_v, W_o, b_o):
    """Build the 8 per-core input maps. Returns (in_maps, b_o_eff)."""
    import ml_dtypes

    f32 = np.float32
    bf16 = ml_dtypes.bfloat16
    masks = np.zeros((4, P, NQ), dtype=bf16)
    for j in range(4):
        for p in range(P):
            masks[j, p, j * P + p :] = 1.0

    in_maps = []
    for c in range(N_CORES):
        b, g = c // 4, c % 4
        heads = [4 * g + i for i in range(4)]
        wq = np.zeros((2, P, E), dtype=bf16)
        wk = np.zeros((2, P, E), dtype=bf16)
        wv = np.zeros((2, P, E), dtype=bf16)
        bq = np.zeros((2, P, 1), dtype=f32)
        bk = np.zeros((2, P, 1), dtype=f32)
        wo = np.zeros((2, P, E), dtype=bf16)

        def batch_layout(wpair):
            # [E, 128] -> [128, EC*128]: partition p = e-row within chunk,
            # columns = (e-chunk, pair-dim) so per-chunk slices are views
            return wpair.reshape(EC, P, P).transpose(1, 0, 2).reshape(P, E)

        for pr in range(2):
            h0, h1 = heads[2 * pr], heads[2 * pr + 1]
            wpair_q = np.concatenate([W_q[h0], W_q[h1]], axis=1) * 0.125
            wpair_k = np.concatenate([W_k[h0], W_k[h1]], axis=1)
            wpair_v = np.concatenate([W_v[h0], W_v[h1]], axis=1)
            wq[pr] = batch_layout(wpair_q).astype(bf16)
            wk[pr] = batch_layout(wpair_k).astype(bf16)
            wv[pr] = batch_layout(wpair_v).astype(bf16)
            bq[pr, :, 0] = np.concatenate([b_q[h0], b_q[h1]]) * 0.125
            bk[pr, :, 0] = np.concatenate([b_k[h0], b_k[h1]])
            wo[pr] = W_o[h0 * D : h0 * D + 2 * D].astype(bf16)
        biases = np.concatenate(
            [bq[0], bk[0], bq[1], bk[1]], axis=1
        )  # [P, 4] f32
        mi = np.concatenate(
            [masks.transpose(1, 0, 2).reshape(P, 4 * NQ), np.eye(P, dtype=bf16)],
            axis=1,
        )  # [P, 4*NQ+P] bf16: masks then identity
        in_maps.append(
            {
                "xT": np.ascontiguousarray(x[b].T).astype(bf16),
                "wq": wq,
                "wk": wk,
                "wv": wv,
                "biases": biases,
                "wo": wo,
                "mi": np.ascontiguousarray(mi.astype(bf16)),
            }
        )
    b_o_eff = (b_v.reshape(-1).astype(f32) @ W_o.astype(f32) + b_o).astype(f32)
    return in_maps, b_o_eff


_PROGRAM = None


def _run(in_maps, trace=False):
    from concourse.bass_utils import run_bass_kernel_spmd

    global _PROGRAM
    if _PROGRAM is None:
        _PROGRAM = _build_program()
    return run_bass_kernel_spmd(
        _PROGRAM, in_maps, core_ids=list(range(N_CORES)), trace=trace
    )


def kernel(x, W_q, b_q, W_k, b_k, W_v, b_v, W_o, b_o, _trace=False, _result_box=None):
    _ensure_axon_hooks()
    args = [np.asarray(a, dtype=np.float32) for a in (x, W_q, b_q, W_k, b_k, W_v, b_v, W_o, b_o)]
    in_maps, b_o_eff = _host_shard(*args)
    res = _run(in_maps, trace=_trace)
    if _result_box is not None:
        _result_box.append(res)
    B = x.shape[0]
    out = np.zeros((B, S, E), dtype=np.float32)
    for c in range(N_CORES):
        out[c // 4] += res.results[c]["out"].astype(np.float32)
    out += b_o_eff
    return out

